# revision 5
# baseline (speedup 1.0000x reference)
"""NaMMAttention Trainium2 kernel: 8-core SPMD, zero collectives.

Sharding: core c handles batch b=c//4 and heads 3*(c%4)..3*(c%4)+2.
Each core computes QKV projection (its head columns, full 2048-token packed
sequence of its batch), RMS QK-norm, dense attention for its 3 (b,h) pairs,
and a partial output projection (its heads' rows of Wout).  The host sums the
4 partial [1536,2048] outputs per batch, adds the output bias, and reshapes.

Layouts: stage 1 runs token-major (stationary = X^T tiles) so the RMS-norm
reduction over head_dim lands on the free axis; Q/K are then PE-transposed to
feature-major for attention.  Attention uses the S^T = K Q^T layout: softmax
normalizer is a ones-vector matmul on PE, rs_k folds into the exp scale, and
1/den is partition-broadcast on GpSimd.
"""

import numpy as np
import ml_dtypes

import concourse.bacc as bacc
import concourse.tile as tile
import concourse.mybir as mybir
from concourse.bass_utils import run_bass_kernel_spmd

F32 = mybir.dt.float32
BF16 = mybir.dt.bfloat16
AF = mybir.ActivationFunctionType
MUL = mybir.AluOpType.mult
ADD = mybir.AluOpType.add

H, D, DIM = 12, 128, 1536
B, LV, LT = 2, 1920, 128
L = LV + LT              # 2048 packed tokens per batch
HC = 3                   # heads per core
G = HC * D               # 384 cols per q/k/v group
NF = DIM // 128          # 12 feature chunks
NT = L // 128            # 16 token tiles (0..14 vid, 15 txt)
QW = 1024                # attention query-chunk width
NQC = L // QW            # 2
EPS = 1e-6
SOFT = 1.0 / float(np.sqrt(D))

_CACHE = {}


def build():
    nc = bacc.Bacc("TRN2", target_bir_lowering=False, debug=False, num_devices=8)

    xT = nc.declare_dram_parameter("xT", [DIM, L], BF16, isOutput=False)
    wv = nc.declare_dram_parameter("wv", [DIM, 3 * G], BF16, isOutput=False)
    wt = nc.declare_dram_parameter("wt", [DIM, 3 * G], BF16, isOutput=False)
    bv = nc.declare_dram_parameter("bv", [1, 3 * G], BF16, isOutput=False)
    bt = nc.declare_dram_parameter("bt", [1, 3 * G], BF16, isOutput=False)
    wov = nc.declare_dram_parameter("wov", [G, DIM], BF16, isOutput=False)
    wot = nc.declare_dram_parameter("wot", [G, DIM], BF16, isOutput=False)
    nqv = nc.declare_dram_parameter("nqv", [D, 1], F32, isOutput=False)
    nkv = nc.declare_dram_parameter("nkv", [D, 1], F32, isOutput=False)
    nqt = nc.declare_dram_parameter("nqt", [D, 1], F32, isOutput=False)
    nkt = nc.declare_dram_parameter("nkt", [D, 1], F32, isOutput=False)
    idb = nc.declare_dram_parameter("idb", [128, 128], BF16, isOutput=False)
    onb = nc.declare_dram_parameter("onb", [128, 1], BF16, isOutput=False)
    onr = nc.declare_dram_parameter("onr", [1, 128], BF16, isOutput=False)
    yT = nc.declare_dram_parameter("yT", [DIM, L], F32, isOutput=True)

    with tile.TileContext(nc) as tc:
        import contextlib
        ctx = contextlib.ExitStack()
        with ctx:
            persist = ctx.enter_context(tc.tile_pool(name="persist", bufs=1))
            wv_sb = [persist.tile([128, 3 * G], BF16, name=f"wv{f}", tag=f"wv{f}")
                     for f in range(NF)]
            wt_sb = [persist.tile([128, 3 * G], BF16, name=f"wt{f}", tag=f"wt{f}")
                     for f in range(NF)]
            for f in range(NF):
                nc.sync.dma_start(out=wv_sb[f][:, :], in_=wv[f * 128:(f + 1) * 128, :])
                nc.sync.dma_start(out=wt_sb[f][:, :], in_=wt[f * 128:(f + 1) * 128, :])
            bv_sb = persist.tile([1, 3 * G], BF16, name="bv", tag="bv")
            bt_sb = persist.tile([1, 3 * G], BF16, name="bt", tag="bt")
            nc.sync.dma_start(out=bv_sb[:, :], in_=bv[:, :])
            nc.sync.dma_start(out=bt_sb[:, :], in_=bt[:, :])
            small = {}
            for nm, prm, shp, dt in (("nqv", nqv, [D, 1], F32), ("nkv", nkv, [D, 1], F32),
                                     ("nqt", nqt, [D, 1], F32), ("nkt", nkt, [D, 1], F32),
                                     ("idb", idb, [128, 128], BF16),
                                     ("onb", onb, [128, 1], BF16),
                                     ("onr", onr, [1, 128], BF16)):
                t = persist.tile(shp, dt, name=nm, tag=nm)
                nc.sync.dma_start(out=t[:, :], in_=prm[:, :])
                small[nm] = t

            QT = persist.tile([128, HC * L], BF16, name="QT", tag="QT")
            KT = persist.tile([128, HC * L], BF16, name="KT", tag="KT")
            VA = persist.tile([128, NT * G], BF16, name="VA", tag="VA")
            AT = persist.tile([128, HC * L], BF16, name="AT", tag="AT")
            RSK = persist.tile([128, NT * HC], F32, name="RSK", tag="RSK")

            xc_pool = ctx.enter_context(tc.tile_pool(name="xc", bufs=14))
            scr_pool = ctx.enter_context(tc.tile_pool(name="scr", bufs=3))
            ss_pool = ctx.enter_context(tc.tile_pool(name="ss", bufs=4))

            # ---------------- stage 1: QKV + QK-norm + transpose ----------------
            with tc.tile_pool(name="ps1", bufs=1, space="PSUM") as ps1:
                for half in range(2):            # 2 x 1024 token columns
                    xcs = []
                    for f in range(NF):
                        xt = xc_pool.tile([128, QW], BF16, name="xc", tag="xc")
                        nc.sync.dma_start(
                            out=xt[:, :],
                            in_=xT[f * 128:(f + 1) * 128, half * QW:(half + 1) * QW])
                        xcs.append(xt)
                    for tl in range(8):
                        t = half * 8 + tl
                        txt = (t == NT - 1)
                        w_sb = wt_sb if txt else wv_sb
                        b_sb = bt_sb if txt else bv_sb
                        nq_ap = small["nqt"] if txt else small["nqv"]
                        nk_ap = small["nkt"] if txt else small["nkv"]

                        pg = [ps1.tile([128, G], F32, name=f"qkv{g}", tag="qkv", bufs=4)
                              for g in range(3)]
                        for f in range(NF):
                            for g in range(3):
                                nc.tensor.matmul(
                                    pg[g][:, :],
                                    lhsT=xcs[f][:, tl * 128:(tl + 1) * 128],
                                    rhs=w_sb[f][:, g * G:(g + 1) * G],
                                    start=(f == 0), stop=False)
                        for g in range(3):
                            nc.tensor.matmul(
                                pg[g][:, :], lhsT=small["onr"][0:1, :],
                                rhs=b_sb[0:1, g * G:(g + 1) * G],
                                start=False, stop=True)

                        # rms statistics for q,k (per head); rs = rsqrt(mean+eps)
                        ss = ss_pool.tile([128, 2 * HC], F32, name="ss", tag="ss")
                        for h in range(HC):
                            sqs = scr_pool.tile([128, 128], BF16, name="sqs",
                                                tag="sqs", bufs=2)
                            nc.scalar.activation(
                                sqs[:, :], pg[0][:, h * 128:(h + 1) * 128],
                                AF.Square, accum_out=ss[:, h:h + 1])
                            sqs2 = scr_pool.tile([128, 128], BF16, name="sqs2",
                                                 tag="sqs", bufs=2)
                            nc.scalar.activation(
                                sqs2[:, :], pg[1][:, h * 128:(h + 1) * 128],
                                AF.Square, accum_out=ss[:, HC + h:HC + h + 1])
                        mm = ss_pool.tile([128, 2 * HC], F32, name="mm", tag="mm")
                        nc.vector.tensor_scalar(mm[:, :], ss[:, :], 1.0 / D, EPS, MUL, ADD)
                        r0 = ss_pool.tile([128, 2 * HC], F32, name="r0", tag="r0")
                        nc.scalar.activation(r0[:, :], mm[:, :], AF.Abs_reciprocal_sqrt)
                        # one Newton step: rs = r0*(1.5 - 0.5*m*r0^2)
                        t1 = ss_pool.tile([128, 2 * HC], F32, name="t1", tag="t1")
                        nc.vector.tensor_mul(t1[:, :], r0[:, :], r0[:, :])
                        t2 = ss_pool.tile([128, 2 * HC], F32, name="t2", tag="t2")
                        nc.vector.tensor_mul(t2[:, :], t1[:, :], mm[:, :])
                        t3 = ss_pool.tile([128, 2 * HC], F32, name="t3", tag="t3")
                        nc.vector.tensor_scalar(t3[:, :], t2[:, :], -0.5, 1.5, MUL, ADD)
                        rs = ss_pool.tile([128, 2 * HC], F32, name="rs", tag="rs")
                        nc.vector.tensor_mul(rs[:, :], r0[:, :], t3[:, :])
                        nc.vector.tensor_scalar(
                            RSK[:, t * HC:(t + 1) * HC], rs[:, HC:2 * HC],
                            SOFT, None, MUL)

                        # v evict (token-major)
                        nc.vector.tensor_copy(VA[:, t * G:(t + 1) * G], pg[2][:, :])

                        # q-hat (scaled by rs_q), k raw; transpose to feature-major
                        qh = scr_pool.tile([128, G], BF16, name="qh", tag="qh")
                        for h in range(HC):
                            nc.vector.tensor_scalar(
                                qh[:, h * 128:(h + 1) * 128],
                                pg[0][:, h * 128:(h + 1) * 128],
                                rs[:, h:h + 1], None, MUL)
                        kh = scr_pool.tile([128, G], BF16, name="kh", tag="kh")
                        nc.vector.tensor_copy(kh[:, :], pg[1][:, :])
                        for h in range(HC):
                            tq = ps1.tile([128, 128], BF16, name="tq", tag="tr", bufs=3)
                            nc.tensor.transpose(
                                tq[:, :], qh[:, h * 128:(h + 1) * 128],
                                small["idb"][:, :])
                            nc.vector.tensor_scalar(
                                QT[:, h * L + t * 128:h * L + (t + 1) * 128],
                                tq[:, :], nq_ap[:, 0:1], None, MUL)
                            tk = ps1.tile([128, 128], BF16, name="tk", tag="tr", bufs=3)
                            nc.tensor.transpose(
                                tk[:, :], kh[:, h * 128:(h + 1) * 128],
                                small["idb"][:, :])
                            nc.vector.tensor_scalar(
                                KT[:, h * L + t * 128:h * L + (t + 1) * 128],
                                tk[:, :], nk_ap[:, 0:1], None, MUL)

            # stage-4 weights arrive while attention runs
            wov_sb = [persist.tile([128, DIM], BF16, name=f"wov{h}", tag=f"wov{h}")
                      for h in range(HC)]
            wot_sb = [persist.tile([128, DIM], BF16, name=f"wot{h}", tag=f"wot{h}")
                      for h in range(HC)]
            for h in range(HC):
                nc.sync.dma_start(out=wov_sb[h][:, :], in_=wov[h * 128:(h + 1) * 128, :])
                nc.sync.dma_start(out=wot_sb[h][:, :], in_=wot[h * 128:(h + 1) * 128, :])

            # ---------------- stage 3: attention ----------------
            pt_pool = ctx.enter_context(tc.tile_pool(name="pt", bufs=4))
            rd_pool = ctx.enter_context(tc.tile_pool(name="rd", bufs=2))
            bcs_pool = ctx.enter_context(tc.tile_pool(name="bcs", bufs=2))
            with tc.tile_pool(name="ps3", bufs=1, space="PSUM") as ps3:
                for h in range(HC):
                    for qc in range(NQC):
                        O = ps3.tile([128, QW], F32, name="O", tag="O", bufs=1)
                        dn = ps3.tile([1, QW], F32, name="dn", tag="dn", bufs=1)
                        for kt in range(NT):
                            S = ps3.tile([128, QW], F32, name="S", tag="S", bufs=2)
                            for x in range(2):
                                nc.tensor.matmul(
                                    S[:, x * 512:(x + 1) * 512],
                                    lhsT=KT[:, h * L + kt * 128:h * L + (kt + 1) * 128],
                                    rhs=QT[:, h * L + qc * QW + x * 512:
                                           h * L + qc * QW + (x + 1) * 512],
                                    start=True, stop=True)
                            PT = pt_pool.tile([128, QW], BF16, name="PT", tag="PT")
                            nc.scalar.activation(
                                PT[:, :], S[:, :], AF.Exp,
                                scale=RSK[:, kt * HC + h:kt * HC + h + 1])
                            for x in range(2):
                                nc.tensor.matmul(
                                    O[:, x * 512:(x + 1) * 512],
                                    lhsT=VA[:, kt * G + h * 128:kt * G + (h + 1) * 128],
                                    rhs=PT[:, x * 512:(x + 1) * 512],
                                    start=(kt == 0), stop=(kt == NT - 1))
                            for x in range(2):
                                nc.tensor.matmul(
                                    dn[:, x * 512:(x + 1) * 512],
                                    lhsT=small["onb"][:, 0:1],
                                    rhs=PT[:, x * 512:(x + 1) * 512],
                                    start=(kt == 0), stop=(kt == NT - 1))
                        rd = rd_pool.tile([1, QW], F32, name="rd", tag="rd")
                        nc.vector.reciprocal(rd[:, :], dn[:, :])
                        bcs = bcs_pool.tile([128, QW], F32, name="bcs", tag="bcs")
                        nc.gpsimd.partition_broadcast(bcs[:, :], rd[:, :])
                        nc.vector.tensor_mul(
                            AT[:, h * L + qc * QW:h * L + (qc + 1) * QW],
                            O[:, :], bcs[:, :])

            # ---------------- stage 4: output projection ----------------
            yev_pool = ctx.enter_context(tc.tile_pool(name="yev", bufs=3))
            with tc.tile_pool(name="ps4", bufs=1, space="PSUM") as ps4:
                for ot in range(NF):
                    for qc in range(4):
                        Y = ps4.tile([128, 512], F32, name="Y", tag="Y", bufs=3)
                        if qc < 3:
                            for hh in range(HC):
                                nc.tensor.matmul(
                                    Y[:, :],
                                    lhsT=wov_sb[hh][:, ot * 128:(ot + 1) * 128],
                                    rhs=AT[:, hh * L + qc * 512:hh * L + (qc + 1) * 512],
                                    start=(hh == 0), stop=(hh == HC - 1))
                        else:
                            for hh in range(HC):
                                nc.tensor.matmul(
                                    Y[:, 0:384],
                                    lhsT=wov_sb[hh][:, ot * 128:(ot + 1) * 128],
                                    rhs=AT[:, hh * L + 1536:hh * L + 1920],
                                    start=(hh == 0), stop=(hh == HC - 1))
                            for hh in range(HC):
                                nc.tensor.matmul(
                                    Y[:, 384:512],
                                    lhsT=wot_sb[hh][:, ot * 128:(ot + 1) * 128],
                                    rhs=AT[:, hh * L + LV:hh * L + L],
                                    start=(hh == 0), stop=(hh == HC - 1))
                        ye = yev_pool.tile([128, 512], F32, name="ye", tag="ye")
                        nc.vector.tensor_copy(ye[:, :], Y[:, :])
                        nc.sync.dma_start(
                            out=yT[ot * 128:(ot + 1) * 128, qc * 512:(qc + 1) * 512],
                            in_=ye[:, :])

    nc.compile()
    return nc


def _prep_inputs(inputs):
    f32 = np.float32
    bf16 = ml_dtypes.bfloat16
    vid = np.asarray(inputs["vid"], f32).reshape(B, LV, DIM)
    txt = np.asarray(inputs["txt"], f32).reshape(B, LT, DIM)
    Wqv = np.asarray(inputs["Wqkv_vid"], f32)
    Wqt = np.asarray(inputs["Wqkv_txt"], f32)
    bqv = np.asarray(inputs["bqkv_vid"], f32)
    bqt = np.asarray(inputs["bqkv_txt"], f32)
    Wov = np.asarray(inputs["Wout_vid"], f32)
    Wot = np.asarray(inputs["Wout_txt"], f32)

    idb = np.eye(128, dtype=bf16)
    onb = np.ones((128, 1), dtype=bf16)
    onr = np.ones((1, 128), dtype=bf16)

    def colslice(Wfull, h0):
        cols = [Wfull[:, g * DIM + h0 * D: g * DIM + (h0 + HC) * D] for g in range(3)]
        return np.ascontiguousarray(np.concatenate(cols, axis=1).astype(bf16))

    def bslice(bfull, h0):
        cols = [bfull[g * DIM + h0 * D: g * DIM + (h0 + HC) * D] for g in range(3)]
        return np.concatenate(cols).reshape(1, 3 * G).astype(bf16)

    in_maps = []
    for c in range(8):
        b = c // 4
        h0 = HC * (c % 4)
        X = np.concatenate([vid[b], txt[b]], axis=0)         # [2048, 1536]
        m = {
            "xT": np.ascontiguousarray(X.T.astype(bf16)),
            "wv": colslice(Wqv, h0),
            "wt": colslice(Wqt, h0),
            "bv": bslice(bqv, h0),
            "bt": bslice(bqt, h0),
            "wov": np.ascontiguousarray(Wov[h0 * D:(h0 + HC) * D, :].astype(bf16)),
            "wot": np.ascontiguousarray(Wot[h0 * D:(h0 + HC) * D, :].astype(bf16)),
            "nqv": np.asarray(inputs["nq_vid"], f32).reshape(D, 1),
            "nkv": np.asarray(inputs["nk_vid"], f32).reshape(D, 1),
            "nqt": np.asarray(inputs["nq_txt"], f32).reshape(D, 1),
            "nkt": np.asarray(inputs["nk_txt"], f32).reshape(D, 1),
            "idb": idb, "onb": onb, "onr": onr,
        }
        in_maps.append(m)
    return in_maps


def kernel(**inputs):
    if "nc" not in _CACHE:
        _CACHE["nc"] = build()
    nc = _CACHE["nc"]
    in_maps = _prep_inputs(inputs)
    res = run_bass_kernel_spmd(nc, in_maps, core_ids=list(range(8)))
    ys = [res.results[c]["yT"] for c in range(8)]

    bout_vid = np.asarray(inputs["bout_vid"], np.float32)
    bout_txt = np.asarray(inputs["bout_txt"], np.float32)
    vid_out = np.empty((B, LV, DIM), np.float32)
    txt_out = np.empty((B, LT, DIM), np.float32)
    for b in range(B):
        Yt = ys[4 * b] + ys[4 * b + 1] + ys[4 * b + 2] + ys[4 * b + 3]
        Y = Yt.T                                             # [2048, 1536]
        vid_out[b] = Y[:LV] + bout_vid
        txt_out[b] = Y[LV:] + bout_txt
    return (vid_out.reshape(B * LV, DIM), txt_out.reshape(B * LT, DIM))


# revision 6
# speedup vs baseline: 1.2049x; 1.2049x over previous
"""NaMMAttention Trainium2 kernel: 8-core SPMD, zero collectives.

Sharding: core c handles batch b=c//4 and heads 3*(c%4)..3*(c%4)+2.
Each core computes QKV projection (its head columns, full 2048-token packed
sequence of its batch), RMS QK-norm, dense attention for its 3 (b,h) pairs,
and a partial output projection (its heads' rows of Wout).  The host sums the
4 partial [1536,2048] outputs per batch, adds the output bias, and reshapes.

Layouts: stage 1 runs token-major (stationary = X^T tiles) so the RMS-norm
reduction over head_dim lands on the free axis; Q/K are then PE-transposed to
feature-major for attention.  Attention uses the S^T = K Q^T layout: softmax
normalizer is a ones-vector matmul on PE, rs_k folds into the exp scale, and
1/den is partition-broadcast on GpSimd.
"""

import numpy as np
import ml_dtypes

import concourse.bacc as bacc
import concourse.tile as tile
import concourse.mybir as mybir
from concourse.bass_utils import run_bass_kernel_spmd

F32 = mybir.dt.float32
BF16 = mybir.dt.bfloat16
AF = mybir.ActivationFunctionType
MUL = mybir.AluOpType.mult
ADD = mybir.AluOpType.add

H, D, DIM = 12, 128, 1536
B, LV, LT = 2, 1920, 128
L = LV + LT              # 2048 packed tokens per batch
HC = 3                   # heads per core
G = HC * D               # 384 cols per q/k/v group
NF = DIM // 128          # 12 feature chunks
NT = L // 128            # 16 token tiles (0..14 vid, 15 txt)
XW = 1024                # stage-1 x-chunk width
QW = 512                 # attention query-chunk width
NQC = L // QW            # 4
EPS = 1e-6
SOFT = 1.0 / float(np.sqrt(D))

_CACHE = {}


def build():
    nc = bacc.Bacc("TRN2", target_bir_lowering=False, debug=False, num_devices=8)

    xT = nc.declare_dram_parameter("xT", [DIM, L], BF16, isOutput=False)
    wv = nc.declare_dram_parameter("wv", [DIM, 3 * G], BF16, isOutput=False)
    wt = nc.declare_dram_parameter("wt", [DIM, 3 * G], BF16, isOutput=False)
    bv = nc.declare_dram_parameter("bv", [1, 3 * G], BF16, isOutput=False)
    bt = nc.declare_dram_parameter("bt", [1, 3 * G], BF16, isOutput=False)
    wov = nc.declare_dram_parameter("wov", [G, DIM], BF16, isOutput=False)
    wot = nc.declare_dram_parameter("wot", [G, DIM], BF16, isOutput=False)
    nqv = nc.declare_dram_parameter("nqv", [D, 1], F32, isOutput=False)
    nkv = nc.declare_dram_parameter("nkv", [D, 1], F32, isOutput=False)
    nqt = nc.declare_dram_parameter("nqt", [D, 1], F32, isOutput=False)
    nkt = nc.declare_dram_parameter("nkt", [D, 1], F32, isOutput=False)
    idb = nc.declare_dram_parameter("idb", [128, 128], BF16, isOutput=False)
    onb = nc.declare_dram_parameter("onb", [128, 1], BF16, isOutput=False)
    onr = nc.declare_dram_parameter("onr", [1, 128], BF16, isOutput=False)
    yT = nc.declare_dram_parameter("yT", [DIM, L], F32, isOutput=True)

    with tile.TileContext(nc) as tc:
        import contextlib
        ctx = contextlib.ExitStack()
        with ctx:
            persist = ctx.enter_context(tc.tile_pool(name="persist", bufs=1))
            wv_sb = [persist.tile([128, 3 * G], BF16, name=f"wv{f}", tag=f"wv{f}")
                     for f in range(NF)]
            wt_sb = [persist.tile([128, 3 * G], BF16, name=f"wt{f}", tag=f"wt{f}")
                     for f in range(NF)]
            bv_sb = persist.tile([1, 3 * G], BF16, name="bv", tag="bv")
            bt_sb = persist.tile([1, 3 * G], BF16, name="bt", tag="bt")
            nc.sync.dma_start(out=bv_sb[:, :], in_=bv[:, :])
            nc.sync.dma_start(out=bt_sb[:, :], in_=bt[:, :])
            small = {}
            for nm, prm, shp, dt in (("nqv", nqv, [D, 1], F32), ("nkv", nkv, [D, 1], F32),
                                     ("nqt", nqt, [D, 1], F32), ("nkt", nkt, [D, 1], F32),
                                     ("idb", idb, [128, 128], BF16),
                                     ("onb", onb, [128, 1], BF16),
                                     ("onr", onr, [1, 128], BF16)):
                t = persist.tile(shp, dt, name=nm, tag=nm)
                nc.sync.dma_start(out=t[:, :], in_=prm[:, :])
                small[nm] = t

            QT = persist.tile([128, HC * L], BF16, name="QT", tag="QT")
            KT = persist.tile([128, HC * L], BF16, name="KT", tag="KT")
            VA = persist.tile([128, NT * G], BF16, name="VA", tag="VA")
            AT = persist.tile([128, HC * L], BF16, name="AT", tag="AT")
            RSK = persist.tile([128, NT * HC], F32, name="RSK", tag="RSK")

            xc_pool = ctx.enter_context(tc.tile_pool(name="xc", bufs=16))
            scr_pool = ctx.enter_context(tc.tile_pool(name="scr", bufs=3))
            ss_pool = ctx.enter_context(tc.tile_pool(name="ss", bufs=4))

            # ---------------- stage 1: QKV + QK-norm + transpose ----------------
            with tc.tile_pool(name="ps1", bufs=1, space="PSUM") as ps1:
                for half in range(2):            # 2 x 1024 token columns
                    xcs = []
                    for f in range(NF):
                        xt = xc_pool.tile([128, XW], BF16, name="xc", tag="xc")
                        nc.sync.dma_start(
                            out=xt[:, :],
                            in_=xT[f * 128:(f + 1) * 128, half * XW:(half + 1) * XW])
                        xcs.append(xt)
                        if half == 0:
                            nc.sync.dma_start(out=wv_sb[f][:, :],
                                              in_=wv[f * 128:(f + 1) * 128, :])
                        else:
                            nc.sync.dma_start(out=wt_sb[f][:, :],
                                              in_=wt[f * 128:(f + 1) * 128, :])
                    for tl in range(8):
                        t = half * 8 + tl
                        txt = (t == NT - 1)
                        w_sb = wt_sb if txt else wv_sb
                        b_sb = bt_sb if txt else bv_sb
                        nq_ap = small["nqt"] if txt else small["nqv"]
                        nk_ap = small["nkt"] if txt else small["nkv"]

                        pg = [ps1.tile([128, G], F32, name=f"qkv{g}", tag="qkv", bufs=4)
                              for g in range(3)]
                        for f in range(NF):
                            for g in range(3):
                                nc.tensor.matmul(
                                    pg[g][:, :],
                                    lhsT=xcs[f][:, tl * 128:(tl + 1) * 128],
                                    rhs=w_sb[f][:, g * G:(g + 1) * G],
                                    start=(f == 0), stop=False)
                        for g in range(3):
                            nc.tensor.matmul(
                                pg[g][:, :], lhsT=small["onr"][0:1, :],
                                rhs=b_sb[0:1, g * G:(g + 1) * G],
                                start=False, stop=True)

                        # rms statistics for q,k (per head); rs = rsqrt(mean+eps)
                        ss = ss_pool.tile([128, 2 * HC], F32, name="ss", tag="ss")
                        for h in range(HC):
                            sqs = scr_pool.tile([128, 128], BF16, name="sqs",
                                                tag="sqs", bufs=2)
                            nc.scalar.activation(
                                sqs[:, :], pg[0][:, h * 128:(h + 1) * 128],
                                AF.Square, accum_out=ss[:, h:h + 1])
                            sqs2 = scr_pool.tile([128, 128], BF16, name="sqs2",
                                                 tag="sqs", bufs=2)
                            nc.scalar.activation(
                                sqs2[:, :], pg[1][:, h * 128:(h + 1) * 128],
                                AF.Square, accum_out=ss[:, HC + h:HC + h + 1])
                        mm = ss_pool.tile([128, 2 * HC], F32, name="mm", tag="mm")
                        nc.vector.tensor_scalar(mm[:, :], ss[:, :], 1.0 / D, EPS, MUL, ADD)
                        r0 = ss_pool.tile([128, 2 * HC], F32, name="r0", tag="r0")
                        nc.scalar.activation(r0[:, :], mm[:, :], AF.Abs_reciprocal_sqrt)
                        # one Newton step: rs = r0*(1.5 - 0.5*m*r0^2)
                        t1 = ss_pool.tile([128, 2 * HC], F32, name="t1", tag="t1")
                        nc.vector.tensor_mul(t1[:, :], r0[:, :], r0[:, :])
                        t2 = ss_pool.tile([128, 2 * HC], F32, name="t2", tag="t2")
                        nc.vector.tensor_mul(t2[:, :], t1[:, :], mm[:, :])
                        t3 = ss_pool.tile([128, 2 * HC], F32, name="t3", tag="t3")
                        nc.vector.tensor_scalar(t3[:, :], t2[:, :], -0.5, 1.5, MUL, ADD)
                        rs = ss_pool.tile([128, 2 * HC], F32, name="rs", tag="rs")
                        nc.vector.tensor_mul(rs[:, :], r0[:, :], t3[:, :])
                        nc.vector.tensor_scalar(
                            RSK[:, t * HC:(t + 1) * HC], rs[:, HC:2 * HC],
                            SOFT, None, MUL)

                        # v evict (token-major)
                        nc.vector.tensor_copy(VA[:, t * G:(t + 1) * G], pg[2][:, :])

                        # q-hat (scaled by rs_q), k raw; transpose to feature-major
                        qh = scr_pool.tile([128, G], BF16, name="qh", tag="qh")
                        for h in range(HC):
                            nc.vector.tensor_scalar(
                                qh[:, h * 128:(h + 1) * 128],
                                pg[0][:, h * 128:(h + 1) * 128],
                                rs[:, h:h + 1], None, MUL)
                        kh = scr_pool.tile([128, G], BF16, name="kh", tag="kh")
                        nc.vector.tensor_copy(kh[:, :], pg[1][:, :])
                        for h in range(HC):
                            tq = ps1.tile([128, 128], BF16, name="tq", tag="tr", bufs=3)
                            nc.tensor.transpose(
                                tq[:, :], qh[:, h * 128:(h + 1) * 128],
                                small["idb"][:, :])
                            nc.vector.tensor_scalar(
                                QT[:, h * L + t * 128:h * L + (t + 1) * 128],
                                tq[:, :], nq_ap[:, 0:1], None, MUL)
                            tk = ps1.tile([128, 128], BF16, name="tk", tag="tr", bufs=3)
                            nc.tensor.transpose(
                                tk[:, :], kh[:, h * 128:(h + 1) * 128],
                                small["idb"][:, :])
                            nc.vector.tensor_scalar(
                                KT[:, h * L + t * 128:h * L + (t + 1) * 128],
                                tk[:, :], nk_ap[:, 0:1], None, MUL)

            # stage-4 weights arrive while attention runs
            wov_sb = [persist.tile([128, DIM], BF16, name=f"wov{h}", tag=f"wov{h}")
                      for h in range(HC)]
            wot_sb = [persist.tile([128, DIM], BF16, name=f"wot{h}", tag=f"wot{h}")
                      for h in range(HC)]
            for h in range(HC):
                nc.sync.dma_start(out=wov_sb[h][:, :], in_=wov[h * 128:(h + 1) * 128, :])
                nc.sync.dma_start(out=wot_sb[h][:, :], in_=wot[h * 128:(h + 1) * 128, :])

            # ---------------- stage 3: attention ----------------
            pt_pool = ctx.enter_context(tc.tile_pool(name="pt", bufs=4))
            rd_pool = ctx.enter_context(tc.tile_pool(name="rd", bufs=2))
            bcs_pool = ctx.enter_context(tc.tile_pool(name="bcs", bufs=2))
            with tc.tile_pool(name="ps3", bufs=1, space="PSUM") as ps3:
                for h in range(HC):
                    for qc in range(NQC):
                        O = ps3.tile([128, QW], F32, name="O", tag="O", bufs=2)
                        dn = ps3.tile([1, QW], F32, name="dn", tag="dn", bufs=2)
                        for kt in range(NT):
                            S = ps3.tile([128, QW], F32, name="S", tag="S", bufs=3)
                            nc.tensor.matmul(
                                S[:, :],
                                lhsT=KT[:, h * L + kt * 128:h * L + (kt + 1) * 128],
                                rhs=QT[:, h * L + qc * QW:h * L + (qc + 1) * QW],
                                start=True, stop=True)
                            PT = pt_pool.tile([128, QW], BF16, name="PT", tag="PT")
                            nc.scalar.activation(
                                PT[:, :], S[:, :], AF.Exp,
                                scale=RSK[:, kt * HC + h:kt * HC + h + 1])
                            nc.tensor.matmul(
                                O[:, :],
                                lhsT=VA[:, kt * G + h * 128:kt * G + (h + 1) * 128],
                                rhs=PT[:, :],
                                start=(kt == 0), stop=(kt == NT - 1))
                            nc.tensor.matmul(
                                dn[:, :],
                                lhsT=small["onb"][:, 0:1],
                                rhs=PT[:, :],
                                start=(kt == 0), stop=(kt == NT - 1))
                        rd = rd_pool.tile([1, QW], F32, name="rd", tag="rd")
                        nc.vector.reciprocal_approx_fast(rd[:, :], dn[:, :])
                        bcs = bcs_pool.tile([128, QW], F32, name="bcs", tag="bcs")
                        nc.gpsimd.partition_broadcast(bcs[:, :], rd[:, :])
                        nc.vector.tensor_mul(
                            AT[:, h * L + qc * QW:h * L + (qc + 1) * QW],
                            O[:, :], bcs[:, :])

            # ---------------- stage 4: output projection ----------------
            yev_pool = ctx.enter_context(tc.tile_pool(name="yev", bufs=3))
            with tc.tile_pool(name="ps4", bufs=1, space="PSUM") as ps4:
                for ot in range(NF):
                    for qc in range(4):
                        Y = ps4.tile([128, 512], F32, name="Y", tag="Y", bufs=3)
                        if qc < 3:
                            for hh in range(HC):
                                nc.tensor.matmul(
                                    Y[:, :],
                                    lhsT=wov_sb[hh][:, ot * 128:(ot + 1) * 128],
                                    rhs=AT[:, hh * L + qc * 512:hh * L + (qc + 1) * 512],
                                    start=(hh == 0), stop=(hh == HC - 1))
                        else:
                            for hh in range(HC):
                                nc.tensor.matmul(
                                    Y[:, 0:384],
                                    lhsT=wov_sb[hh][:, ot * 128:(ot + 1) * 128],
                                    rhs=AT[:, hh * L + 1536:hh * L + 1920],
                                    start=(hh == 0), stop=(hh == HC - 1))
                            for hh in range(HC):
                                nc.tensor.matmul(
                                    Y[:, 384:512],
                                    lhsT=wot_sb[hh][:, ot * 128:(ot + 1) * 128],
                                    rhs=AT[:, hh * L + LV:hh * L + L],
                                    start=(hh == 0), stop=(hh == HC - 1))
                        ye = yev_pool.tile([128, 512], F32, name="ye", tag="ye")
                        nc.vector.tensor_copy(ye[:, :], Y[:, :])
                        nc.sync.dma_start(
                            out=yT[ot * 128:(ot + 1) * 128, qc * 512:(qc + 1) * 512],
                            in_=ye[:, :])

    nc.compile()
    return nc


def _prep_inputs(inputs):
    f32 = np.float32
    bf16 = ml_dtypes.bfloat16
    vid = np.asarray(inputs["vid"], f32).reshape(B, LV, DIM)
    txt = np.asarray(inputs["txt"], f32).reshape(B, LT, DIM)
    Wqv = np.asarray(inputs["Wqkv_vid"], f32)
    Wqt = np.asarray(inputs["Wqkv_txt"], f32)
    bqv = np.asarray(inputs["bqkv_vid"], f32)
    bqt = np.asarray(inputs["bqkv_txt"], f32)
    Wov = np.asarray(inputs["Wout_vid"], f32)
    Wot = np.asarray(inputs["Wout_txt"], f32)

    idb = np.eye(128, dtype=bf16)
    onb = np.ones((128, 1), dtype=bf16)
    onr = np.ones((1, 128), dtype=bf16)

    def colslice(Wfull, h0):
        cols = [Wfull[:, g * DIM + h0 * D: g * DIM + (h0 + HC) * D] for g in range(3)]
        return np.ascontiguousarray(np.concatenate(cols, axis=1).astype(bf16))

    def bslice(bfull, h0):
        cols = [bfull[g * DIM + h0 * D: g * DIM + (h0 + HC) * D] for g in range(3)]
        return np.concatenate(cols).reshape(1, 3 * G).astype(bf16)

    in_maps = []
    for c in range(8):
        b = c // 4
        h0 = HC * (c % 4)
        X = np.concatenate([vid[b], txt[b]], axis=0)         # [2048, 1536]
        m = {
            "xT": np.ascontiguousarray(X.T.astype(bf16)),
            "wv": colslice(Wqv, h0),
            "wt": colslice(Wqt, h0),
            "bv": bslice(bqv, h0),
            "bt": bslice(bqt, h0),
            "wov": np.ascontiguousarray(Wov[h0 * D:(h0 + HC) * D, :].astype(bf16)),
            "wot": np.ascontiguousarray(Wot[h0 * D:(h0 + HC) * D, :].astype(bf16)),
            "nqv": np.asarray(inputs["nq_vid"], f32).reshape(D, 1),
            "nkv": np.asarray(inputs["nk_vid"], f32).reshape(D, 1),
            "nqt": np.asarray(inputs["nq_txt"], f32).reshape(D, 1),
            "nkt": np.asarray(inputs["nk_txt"], f32).reshape(D, 1),
            "idb": idb, "onb": onb, "onr": onr,
        }
        in_maps.append(m)
    return in_maps


def kernel(**inputs):
    if "nc" not in _CACHE:
        _CACHE["nc"] = build()
    nc = _CACHE["nc"]
    in_maps = _prep_inputs(inputs)
    res = run_bass_kernel_spmd(nc, in_maps, core_ids=list(range(8)))
    ys = [res.results[c]["yT"] for c in range(8)]

    bout_vid = np.asarray(inputs["bout_vid"], np.float32)
    bout_txt = np.asarray(inputs["bout_txt"], np.float32)
    vid_out = np.empty((B, LV, DIM), np.float32)
    txt_out = np.empty((B, LT, DIM), np.float32)
    for b in range(B):
        Yt = ys[4 * b] + ys[4 * b + 1] + ys[4 * b + 2] + ys[4 * b + 3]
        Y = Yt.T                                             # [2048, 1536]
        vid_out[b] = Y[:LV] + bout_vid
        txt_out[b] = Y[LV:] + bout_txt
    return (vid_out.reshape(B * LV, DIM), txt_out.reshape(B * LT, DIM))


# revision 7
# speedup vs baseline: 1.3146x; 1.0911x over previous
"""NaMMAttention Trainium2 kernel: 8-core SPMD, zero collectives.

Sharding: core c handles batch b=c//4 and heads 3*(c%4)..3*(c%4)+2.
Each core computes QKV projection (its head columns, full 2048-token packed
sequence of its batch), RMS QK-norm, dense attention for its 3 (b,h) pairs,
and a partial output projection (its heads' rows of Wout).  The host sums the
4 partial [1536,2048] outputs per batch, adds the output bias, and reshapes.

Layouts: stage 1 runs token-major (stationary = X^T tiles) so the RMS-norm
reduction over head_dim lands on the free axis; Q/K are then PE-transposed to
feature-major for attention.  Attention uses the S^T = K Q^T layout: softmax
normalizer is a ones-vector matmul on PE, rs_k folds into the exp scale, and
1/den is partition-broadcast on GpSimd.
"""

import numpy as np
import ml_dtypes

import concourse.bacc as bacc
import concourse.tile as tile
import concourse.mybir as mybir
from concourse.bass_utils import run_bass_kernel_spmd

F32 = mybir.dt.float32
BF16 = mybir.dt.bfloat16
AF = mybir.ActivationFunctionType
MUL = mybir.AluOpType.mult
ADD = mybir.AluOpType.add

H, D, DIM = 12, 128, 1536
B, LV, LT = 2, 1920, 128
L = LV + LT              # 2048 packed tokens per batch
HC = 3                   # heads per core
G = HC * D               # 384 cols per q/k/v group
NF = DIM // 128          # 12 feature chunks
NT = L // 128            # 16 token tiles (0..14 vid, 15 txt)
XW = 1024                # stage-1 x-chunk width
QW = 512                 # attention query-chunk width
NQC = L // QW            # 4
EPS = 1e-6
SOFT = 1.0 / float(np.sqrt(D))

_CACHE = {}


def build(with_bias=True):
    nc = bacc.Bacc("TRN2", target_bir_lowering=False, debug=False, num_devices=8)

    xT = nc.declare_dram_parameter("xT", [DIM, L], BF16, isOutput=False)
    wv = nc.declare_dram_parameter("wv", [DIM, 3 * G], BF16, isOutput=False)
    wt = nc.declare_dram_parameter("wt", [DIM, 3 * G], BF16, isOutput=False)
    bv = nc.declare_dram_parameter("bv", [1, 3 * G], BF16, isOutput=False)
    bt = nc.declare_dram_parameter("bt", [1, 3 * G], BF16, isOutput=False)
    wov = nc.declare_dram_parameter("wov", [G, DIM], BF16, isOutput=False)
    wot = nc.declare_dram_parameter("wot", [G, DIM], BF16, isOutput=False)
    nqv = nc.declare_dram_parameter("nqv", [D, 1], F32, isOutput=False)
    nkv = nc.declare_dram_parameter("nkv", [D, 1], F32, isOutput=False)
    nqt = nc.declare_dram_parameter("nqt", [D, 1], F32, isOutput=False)
    nkt = nc.declare_dram_parameter("nkt", [D, 1], F32, isOutput=False)
    idb = nc.declare_dram_parameter("idb", [128, 128], BF16, isOutput=False)
    onb = nc.declare_dram_parameter("onb", [128, 1], BF16, isOutput=False)
    onr = nc.declare_dram_parameter("onr", [1, 128], BF16, isOutput=False)
    yT = nc.declare_dram_parameter("yT", [DIM, L], F32, isOutput=True)

    with tile.TileContext(nc) as tc:
        import contextlib
        ctx = contextlib.ExitStack()
        with ctx:
            persist = ctx.enter_context(tc.tile_pool(name="persist", bufs=1))
            wv_sb = [persist.tile([128, 3 * G], BF16, name=f"wv{f}", tag=f"wv{f}")
                     for f in range(NF)]
            wt_sb = [persist.tile([128, 3 * G], BF16, name=f"wt{f}", tag=f"wt{f}")
                     for f in range(NF)]
            bv_sb = persist.tile([1, 3 * G], BF16, name="bv", tag="bv")
            bt_sb = persist.tile([1, 3 * G], BF16, name="bt", tag="bt")
            nc.sync.dma_start(out=bv_sb[:, :], in_=bv[:, :])
            nc.sync.dma_start(out=bt_sb[:, :], in_=bt[:, :])
            small = {}
            for nm, prm, shp, dt in (("nqv", nqv, [D, 1], F32), ("nkv", nkv, [D, 1], F32),
                                     ("nqt", nqt, [D, 1], F32), ("nkt", nkt, [D, 1], F32),
                                     ("idb", idb, [128, 128], BF16),
                                     ("onb", onb, [128, 1], BF16),
                                     ("onr", onr, [1, 128], BF16)):
                t = persist.tile(shp, dt, name=nm, tag=nm)
                nc.sync.dma_start(out=t[:, :], in_=prm[:, :])
                small[nm] = t

            QT = persist.tile([128, HC * L], BF16, name="QT", tag="QT")
            KT = persist.tile([128, HC * L], BF16, name="KT", tag="KT")
            VA = persist.tile([128, NT * G], BF16, name="VA", tag="VA")
            AT = persist.tile([128, HC * L], BF16, name="AT", tag="AT")
            RSK = persist.tile([128, NT * HC], F32, name="RSK", tag="RSK")

            xc_pool = ctx.enter_context(tc.tile_pool(name="xc", bufs=16))
            scr_pool = ctx.enter_context(tc.tile_pool(name="scr", bufs=3))
            ss_pool = ctx.enter_context(tc.tile_pool(name="ss", bufs=4))

            # ---------------- stage 1: QKV + QK-norm + transpose ----------------
            with tc.tile_pool(name="ps1", bufs=1, space="PSUM") as ps1:
                for half in range(2):            # 2 x 1024 token columns
                    xcs = []
                    for f in range(NF):
                        xt = xc_pool.tile([128, XW], BF16, name="xc", tag="xc")
                        nc.sync.dma_start(
                            out=xt[:, :],
                            in_=xT[f * 128:(f + 1) * 128, half * XW:(half + 1) * XW])
                        xcs.append(xt)
                        if half == 0:
                            nc.sync.dma_start(out=wv_sb[f][:, :],
                                              in_=wv[f * 128:(f + 1) * 128, :])
                        else:
                            nc.sync.dma_start(out=wt_sb[f][:, :],
                                              in_=wt[f * 128:(f + 1) * 128, :])
                    for tl in range(8):
                        t = half * 8 + tl
                        txt = (t == NT - 1)
                        w_sb = wt_sb if txt else wv_sb
                        b_sb = bt_sb if txt else bv_sb
                        nq_ap = small["nqt"] if txt else small["nqv"]
                        nk_ap = small["nkt"] if txt else small["nkv"]

                        pg = [ps1.tile([128, G], F32, name=f"qkv{g}", tag="qkv", bufs=4)
                              for g in range(3)]
                        for f in range(NF):
                            for g in range(3):
                                nc.tensor.matmul(
                                    pg[g][:, :],
                                    lhsT=xcs[f][:, tl * 128:(tl + 1) * 128],
                                    rhs=w_sb[f][:, g * G:(g + 1) * G],
                                    start=(f == 0),
                                    stop=(not with_bias and f == NF - 1))
                        if with_bias:
                            for g in range(3):
                                nc.tensor.matmul(
                                    pg[g][:, :], lhsT=small["onr"][0:1, :],
                                    rhs=b_sb[0:1, g * G:(g + 1) * G],
                                    start=False, stop=True)

                        # rms statistics for q,k (per head); rs = rsqrt(mean+eps)
                        ss = ss_pool.tile([128, 2 * HC], F32, name="ss", tag="ss")
                        for h in range(HC):
                            sqs = scr_pool.tile([128, 128], BF16, name="sqs",
                                                tag="sqs", bufs=2)
                            nc.scalar.activation(
                                sqs[:, :], pg[0][:, h * 128:(h + 1) * 128],
                                AF.Square, accum_out=ss[:, h:h + 1])
                            sqs2 = scr_pool.tile([128, 128], BF16, name="sqs2",
                                                 tag="sqs", bufs=2)
                            nc.scalar.activation(
                                sqs2[:, :], pg[1][:, h * 128:(h + 1) * 128],
                                AF.Square, accum_out=ss[:, HC + h:HC + h + 1])
                        mm = ss_pool.tile([128, 2 * HC], F32, name="mm", tag="mm")
                        nc.vector.tensor_scalar(mm[:, :], ss[:, :], 1.0 / D, EPS, MUL, ADD)
                        r0 = ss_pool.tile([128, 2 * HC], F32, name="r0", tag="r0")
                        nc.scalar.activation(r0[:, :], mm[:, :], AF.Abs_reciprocal_sqrt)
                        # one Newton step: rs = r0*(1.5 - 0.5*m*r0^2)
                        t1 = ss_pool.tile([128, 2 * HC], F32, name="t1", tag="t1")
                        nc.vector.tensor_mul(t1[:, :], r0[:, :], r0[:, :])
                        t2 = ss_pool.tile([128, 2 * HC], F32, name="t2", tag="t2")
                        nc.vector.tensor_mul(t2[:, :], t1[:, :], mm[:, :])
                        t3 = ss_pool.tile([128, 2 * HC], F32, name="t3", tag="t3")
                        nc.vector.tensor_scalar(t3[:, :], t2[:, :], -0.5, 1.5, MUL, ADD)
                        rs = ss_pool.tile([128, 2 * HC], F32, name="rs", tag="rs")
                        nc.vector.tensor_mul(rs[:, :], r0[:, :], t3[:, :])
                        nc.vector.tensor_scalar(
                            RSK[:, t * HC:(t + 1) * HC], rs[:, HC:2 * HC],
                            SOFT, None, MUL)

                        # v evict (token-major)
                        nc.vector.tensor_copy(VA[:, t * G:(t + 1) * G], pg[2][:, :])

                        # q-hat (scaled by rs_q), k raw; transpose to feature-major
                        qh = scr_pool.tile([128, G], BF16, name="qh", tag="qh")
                        for h in range(HC):
                            nc.vector.tensor_scalar(
                                qh[:, h * 128:(h + 1) * 128],
                                pg[0][:, h * 128:(h + 1) * 128],
                                rs[:, h:h + 1], None, MUL)
                        kh = scr_pool.tile([128, G], BF16, name="kh", tag="kh")
                        nc.vector.tensor_copy(kh[:, :], pg[1][:, :])
                        for h in range(HC):
                            tq = ps1.tile([128, 128], BF16, name="tq", tag="tr", bufs=3)
                            nc.tensor.transpose(
                                tq[:, :], qh[:, h * 128:(h + 1) * 128],
                                small["idb"][:, :])
                            nc.vector.tensor_scalar(
                                QT[:, h * L + t * 128:h * L + (t + 1) * 128],
                                tq[:, :], nq_ap[:, 0:1], None, MUL)
                            tk = ps1.tile([128, 128], BF16, name="tk", tag="tr", bufs=3)
                            nc.tensor.transpose(
                                tk[:, :], kh[:, h * 128:(h + 1) * 128],
                                small["idb"][:, :])
                            nc.vector.tensor_scalar(
                                KT[:, h * L + t * 128:h * L + (t + 1) * 128],
                                tk[:, :], nk_ap[:, 0:1], None, MUL)

            # stage-4 weights arrive while attention runs
            wov_sb = [persist.tile([128, DIM], BF16, name=f"wov{h}", tag=f"wov{h}")
                      for h in range(HC)]
            wot_sb = [persist.tile([128, DIM], BF16, name=f"wot{h}", tag=f"wot{h}")
                      for h in range(HC)]
            for h in range(HC):
                nc.sync.dma_start(out=wov_sb[h][:, :], in_=wov[h * 128:(h + 1) * 128, :])
                nc.sync.dma_start(out=wot_sb[h][:, :], in_=wot[h * 128:(h + 1) * 128, :])

            # ---------------- stage 3+4: attention + output projection ----------------
            pt_pool = ctx.enter_context(tc.tile_pool(name="pt", bufs=4))
            rd_pool = ctx.enter_context(tc.tile_pool(name="rd", bufs=2))
            bcs_pool = ctx.enter_context(tc.tile_pool(name="bcs", bufs=2))
            yev_pool = ctx.enter_context(tc.tile_pool(name="yev", bufs=3))
            with tc.tile_pool(name="ps3", bufs=1, space="PSUM") as ps3:
                for qc in range(NQC):
                    for h in range(HC):
                        O = ps3.tile([128, QW], F32, name="O", tag="O", bufs=2)
                        dn = ps3.tile([1, QW], F32, name="dn", tag="dn", bufs=1)
                        for kt in range(NT):
                            S = ps3.tile([128, QW], F32, name="S", tag="S", bufs=3)
                            nc.tensor.matmul(
                                S[:, :],
                                lhsT=KT[:, h * L + kt * 128:h * L + (kt + 1) * 128],
                                rhs=QT[:, h * L + qc * QW:h * L + (qc + 1) * QW],
                                start=True, stop=True)
                            PT = pt_pool.tile([128, QW], BF16, name="PT", tag="PT")
                            nc.scalar.activation(
                                PT[:, :], S[:, :], AF.Exp,
                                scale=RSK[:, kt * HC + h:kt * HC + h + 1])
                            nc.tensor.matmul(
                                O[:, :],
                                lhsT=VA[:, kt * G + h * 128:kt * G + (h + 1) * 128],
                                rhs=PT[:, :],
                                start=(kt == 0), stop=(kt == NT - 1))
                            nc.tensor.matmul(
                                dn[:, :],
                                lhsT=small["onb"][:, 0:1],
                                rhs=PT[:, :],
                                start=(kt == 0), stop=(kt == NT - 1))
                        rd = rd_pool.tile([1, QW], F32, name="rd", tag="rd")
                        nc.vector.reciprocal_approx_fast(rd[:, :], dn[:, :])
                        bcs = bcs_pool.tile([128, QW], F32, name="bcs", tag="bcs")
                        nc.gpsimd.partition_broadcast(bcs[:, :], rd[:, :])
                        nc.vector.tensor_mul(
                            AT[:, h * L + qc * QW:h * L + (qc + 1) * QW],
                            O[:, :], bcs[:, :])
                    # output projection for this query chunk
                    for ot in range(NF):
                        Y = ps3.tile([128, QW], F32, name="Y", tag="Y", bufs=2)
                        if qc < 3:
                            for hh in range(HC):
                                nc.tensor.matmul(
                                    Y[:, :],
                                    lhsT=wov_sb[hh][:, ot * 128:(ot + 1) * 128],
                                    rhs=AT[:, hh * L + qc * 512:hh * L + (qc + 1) * 512],
                                    start=(hh == 0), stop=(hh == HC - 1))
                        else:
                            for hh in range(HC):
                                nc.tensor.matmul(
                                    Y[:, 0:384],
                                    lhsT=wov_sb[hh][:, ot * 128:(ot + 1) * 128],
                                    rhs=AT[:, hh * L + 1536:hh * L + 1920],
                                    start=(hh == 0), stop=(hh == HC - 1))
                            for hh in range(HC):
                                nc.tensor.matmul(
                                    Y[:, 384:512],
                                    lhsT=wot_sb[hh][:, ot * 128:(ot + 1) * 128],
                                    rhs=AT[:, hh * L + LV:hh * L + L],
                                    start=(hh == 0), stop=(hh == HC - 1))
                        ye = yev_pool.tile([128, QW], F32, name="ye", tag="ye")
                        nc.vector.tensor_copy(ye[:, :], Y[:, :])
                        nc.sync.dma_start(
                            out=yT[ot * 128:(ot + 1) * 128, qc * 512:(qc + 1) * 512],
                            in_=ye[:, :])

    nc.compile()
    return nc


def _prep_inputs(inputs):
    f32 = np.float32
    bf16 = ml_dtypes.bfloat16
    vid = np.asarray(inputs["vid"], f32).reshape(B, LV, DIM)
    txt = np.asarray(inputs["txt"], f32).reshape(B, LT, DIM)
    Wqv = np.asarray(inputs["Wqkv_vid"], f32)
    Wqt = np.asarray(inputs["Wqkv_txt"], f32)
    bqv = np.asarray(inputs["bqkv_vid"], f32)
    bqt = np.asarray(inputs["bqkv_txt"], f32)
    Wov = np.asarray(inputs["Wout_vid"], f32)
    Wot = np.asarray(inputs["Wout_txt"], f32)

    idb = np.eye(128, dtype=bf16)
    onb = np.ones((128, 1), dtype=bf16)
    onr = np.ones((1, 128), dtype=bf16)

    def colslice(Wfull, h0):
        cols = [Wfull[:, g * DIM + h0 * D: g * DIM + (h0 + HC) * D] for g in range(3)]
        return np.ascontiguousarray(np.concatenate(cols, axis=1).astype(bf16))

    def bslice(bfull, h0):
        cols = [bfull[g * DIM + h0 * D: g * DIM + (h0 + HC) * D] for g in range(3)]
        return np.concatenate(cols).reshape(1, 3 * G).astype(bf16)

    in_maps = []
    for c in range(8):
        b = c // 4
        h0 = HC * (c % 4)
        X = np.concatenate([vid[b], txt[b]], axis=0)         # [2048, 1536]
        m = {
            "xT": np.ascontiguousarray(X.T.astype(bf16)),
            "wv": colslice(Wqv, h0),
            "wt": colslice(Wqt, h0),
            "bv": bslice(bqv, h0),
            "bt": bslice(bqt, h0),
            "wov": np.ascontiguousarray(Wov[h0 * D:(h0 + HC) * D, :].astype(bf16)),
            "wot": np.ascontiguousarray(Wot[h0 * D:(h0 + HC) * D, :].astype(bf16)),
            "nqv": np.asarray(inputs["nq_vid"], f32).reshape(D, 1),
            "nkv": np.asarray(inputs["nk_vid"], f32).reshape(D, 1),
            "nqt": np.asarray(inputs["nq_txt"], f32).reshape(D, 1),
            "nkt": np.asarray(inputs["nk_txt"], f32).reshape(D, 1),
            "idb": idb, "onb": onb, "onr": onr,
        }
        in_maps.append(m)
    return in_maps


def kernel(**inputs):
    with_bias = bool(np.any(np.asarray(inputs["bqkv_vid"]))
                     or np.any(np.asarray(inputs["bqkv_txt"])))
    key = ("nc", with_bias)
    if key not in _CACHE:
        _CACHE[key] = build(with_bias)
    nc = _CACHE[key]
    _CACHE["nc"] = nc
    in_maps = _prep_inputs(inputs)
    res = run_bass_kernel_spmd(nc, in_maps, core_ids=list(range(8)))
    ys = [res.results[c]["yT"] for c in range(8)]

    bout_vid = np.asarray(inputs["bout_vid"], np.float32)
    bout_txt = np.asarray(inputs["bout_txt"], np.float32)
    vid_out = np.empty((B, LV, DIM), np.float32)
    txt_out = np.empty((B, LT, DIM), np.float32)
    for b in range(B):
        Yt = ys[4 * b] + ys[4 * b + 1] + ys[4 * b + 2] + ys[4 * b + 3]
        Y = Yt.T                                             # [2048, 1536]
        vid_out[b] = Y[:LV] + bout_vid
        txt_out[b] = Y[LV:] + bout_txt
    return (vid_out.reshape(B * LV, DIM), txt_out.reshape(B * LT, DIM))
